# revision 23
# baseline (speedup 1.0000x reference)
"""Trainium2 Bass kernel for ClassicalSelfAttention.

Math (per batch b):
    q = (x @ w_q.T) @ R ; k = (x @ w_k.T) @ Ent ; v = x @ w_v.T
    per head h: out_h = softmax(q_h @ k_h.T / sqrt(64)) @ v_h
    out[b, s, h*64+d]

Sharding: 8 cores, core i handles batch b = i // 4 and the adjacent head
pair m = i % 4 (global heads 2m, 2m+1 -> output columns 128m..128m+128).
No inter-core communication.

Design notes (v3):
  - ScalarE exp is the hard floor: 2 heads x 4096^2 scores / 128 lanes at
    1 elem/cycle/lane @ 1.2GHz = 218.5us + ~0.16us per ACTIVATE (171 of
    them) ~= 246us.  Everything else is scheduled to hide behind it.
  - ALL matmul operands fp16.  Combined projection weights (w_q.T @ R
    column slice etc.) are computed on the host in f32 and shipped fp16.
  - x^T lands via 32 chunk DMAs round-robined over three queues
    (gpsimd/vector/sync) so block 0 is resident ~3us in and the whole
    tensor by ~9us; the serial single-queue DMA stream previously
    delayed the first ACTIVATE to 25+us.
  - one global score-slot stream: slot g = 64*qb + 2*kt + h; a
    [128, 3, 512] PSUM tile holds 3 slots, one ACTIVATE each, spanning
    q-block boundaries (no per-block partial flush).
  - PV matmuls are drained lazily, only when the exp-tile ring (EXB=8
    tiles) is about to evict a tile.  This shifts ~20 slots of PV work
    out of the PE-crowded q-block 0 (which also runs all K/V
    projections) into later q-blocks' PE slack, keeping the exp stream
    fed from both ends.
  - the score PSUM pool is a strict 2-buffer ring; ANY extra tile
    allocation flips bank parity and stalls the exp stream ~1us.  K/V
    projections therefore insert in PAIRS (parity preserved, real
    dependencies gate every bank reuse).  The per-q-block Q prefetch is
    a single tile, so it is followed by a dummy tile carrying a guard
    copy that reads the just-written exp tile -- restoring parity with a
    real dependency on the in-flight ACTIVATE (no race on bank reuse).
  - PV accumulators are two persistent PSUM banks (one per head); the
    end-of-block transpose writes back into the same banks after the
    copy-out, so 6+2 banks cover everything.
"""

import sys

if "/opt/trn_rl_repo" not in sys.path:
    sys.path.insert(0, "/opt/trn_rl_repo")

import numpy as np

import concourse.bass as bass  # noqa: F401  (engine namespaces live on nc)
import concourse.mybir as mybir
import concourse.tile as tile
from concourse import bacc
from concourse.bass_utils import run_bass_kernel_spmd
from concourse.masks import make_identity

F32 = mybir.dt.float32
F16 = mybir.dt.float16
EXPF = mybir.ActivationFunctionType.Exp

E = 512
D = 64
PAIR = 128  # 2 heads x 64 dims per core
N_CORES = 8


def build_attention_nc(S=4096, exb=8, tail_lag=6):
    """Build the single-core Bass program (SPMD: every core runs this)."""
    EC = E // 128  # e-chunks (contraction over E)
    ST = S // 128  # k-tiles
    QB = S // 512  # query blocks
    NSLOT = 3  # score slots (one (kt, h) each) per PSUM tile
    NS = 2 * ST  # slots per q-block
    TOT = QB * NS  # total score slots
    EXB = exb  # exp-tile ring depth

    nc = bacc.Bacc("TRN2", target_bir_lowering=False, debug=False)

    xT_d = nc.dram_tensor("xT", [E, S], F16, kind="ExternalInput")
    # weights host-packed chunk-major: [128, c*128+f] = w[c*128+p, f]
    wqc_d = nc.dram_tensor("wqc", [128, EC * PAIR], F16, kind="ExternalInput")
    wkc_d = nc.dram_tensor("wkc", [128, EC * PAIR], F16, kind="ExternalInput")
    wvT_d = nc.dram_tensor("wvT", [128, EC * PAIR], F16, kind="ExternalInput")
    out_d = nc.dram_tensor("out", [S, PAIR], F32, kind="ExternalOutput")

    with tile.TileContext(nc) as tc:
        with tc.tile_pool(name="persist", bufs=1) as PST, tc.tile_pool(
            name="pv_persist", bufs=1, space="PSUM"
        ) as PVQ:
            # persistent PV accumulators, one per head; reused by every
            # q-block (WAR deps via the tile framework).  After the copy-out
            # the PE transposes write back INTO these banks (viewed as
            # [128, 4, 65]) so no extra PSUM is needed for the transpose.
            pv_ps = [PVQ.tile([128, 512], F32, name=f"pv_h{h}") for h in range(2)]
            xT_sb = PST.tile([128, EC, S], F16)
            kTb = [PST.tile([128, 512], F16, name=f"kT_{b}") for b in range(QB)]
            qTb = [PST.tile([128, 512], F16, name=f"qT_{b}") for b in range(QB)]
            # V' per k-chunk: [1 | V_h0 (64) | V_h1 (64) | 1] -- ones at the
            # OUTER columns so the projection lands with a single copy and
            # both heads' lhsT slices stay contiguous (h0: 0:65, h1: 65:130)
            vb = [PST.tile([128, 4, 130], F16, name=f"v_{b}") for b in range(QB)]
            out_sb = PST.tile([128, ST, PAIR], F32)
            ident = PST.tile([128, 128], F32)
            wqc_sb = PST.tile([128, EC, PAIR], F16)
            wkc_sb = PST.tile([128, EC, PAIR], F16)
            wvT_sb = PST.tile([128, EC, PAIR], F16)
            make_identity(nc, ident[:])
            # memset can't target fp16; stage in fp32 and convert-copy.
            # ones_f32 is oversized to double as the PE warm-up operand.
            ones_f32 = PST.tile([128, 512], F32)
            nc.vector.memset(ones_f32[:], 1.0)
            ones16 = PST.tile([128, 512], F16)
            nc.vector.tensor_copy(ones16[:], ones_f32[:])

            # ---------------- DMAs ---------------------------------------
            # HBM bandwidth (~358GB/s) is shared across all queues, so the
            # SERVICE ORDER decides when the first projections can start.
            # Priority: block-0 chunks + all three weights first (block 0 +
            # wkc+wqc+wvT = 0.9MB ~ 2.7us), then block 1, then the rest.
            # Weights are host-packed chunk-major ([128, EC*128]) so their
            # DMA moves 1KB contiguous lines instead of 256B.
            def dma_weight(q, dst, src):
                q.dma_start(
                    out=dst[:], in_=src[:].rearrange("p (c f) -> p c f", c=EC)
                )

            def dma_chunk(q, b, c):
                bs = slice(512 * b, 512 * (b + 1))
                sl = slice(128 * c, 128 * (c + 1))
                q.dma_start(out=xT_sb[:, c, bs], in_=xT_d[sl, bs])

            dma_chunk(nc.gpsimd, 0, 0)
            dma_weight(nc.sync, wkc_sb, wkc_d)
            dma_chunk(nc.scalar, 0, 1)
            dma_chunk(nc.gpsimd, 0, 2)
            dma_chunk(nc.sync, 0, 3)
            dma_weight(nc.scalar, wqc_sb, wqc_d)
            dma_weight(nc.gpsimd, wvT_sb, wvT_d)
            # block 1 next, then blocks 2..QB-1 round-robin on gpsimd/sync
            dma_chunk(nc.sync, 1, 0)
            dma_chunk(nc.scalar, 1, 1)
            dma_chunk(nc.gpsimd, 1, 2)
            dma_chunk(nc.scalar, 1, 3)
            di = 0
            for b in range(2, QB):
                for c in range(EC):
                    dma_chunk([nc.gpsimd, nc.sync][di % 2], b, c)
                    di += 1
            # V' ones columns never change: write them once up front
            for b in range(QB):
                nc.vector.tensor_copy(vb[b][:, :, 0:1], ones_f32[:, 0:4])
                nc.vector.tensor_copy(vb[b][:, :, 129:130], ones_f32[:, 0:4])

            # ---------------- attention main loop -----------------------
            with (
                tc.tile_pool(name="sc_ps", bufs=2, space="PSUM") as SC,
                tc.tile_pool(name="exp_sb", bufs=EXB) as EX,
                tc.tile_pool(name="nrm_sb", bufs=2) as NRM,
            ):
                def emit_kqT(b, wc, dst, kind):
                    ps = SC.tile([128, NSLOT, 512], F32, tag="sc", name=f"{kind}ps_{b}")
                    bs = slice(512 * b, 512 * (b + 1))
                    for c in range(EC):
                        nc.tensor.matmul(
                            ps[:, 0, :],
                            lhsT=wc[:, c, :],
                            rhs=xT_sb[:, c, bs],
                            start=(c == 0),
                            stop=(c == EC - 1),
                        )
                    nc.vector.tensor_copy(dst[:], ps[:, 0, :])

                def emit_v(b):
                    ps = SC.tile([128, NSLOT, 512], F32, tag="sc", name=f"vps_{b}")
                    view = ps[:, 0, :].rearrange("p (j n) -> p j n", j=4)
                    for jj in range(4):
                        j = 4 * b + jj
                        for c in range(EC):
                            nc.tensor.matmul(
                                view[:, jj, :],
                                lhsT=xT_sb[:, c, 128 * j : 128 * (j + 1)],
                                rhs=wvT_sb[:, c, :],
                                start=(c == 0),
                                stop=(c == EC - 1),
                            )
                    nc.vector.tensor_copy(vb[b][:, :, 1:129], view[:, :, 0:128])

                # PE warm-up during the ~6us DMA-latency window: the HAM
                # clock gate needs ~3.4us of sustained activity to release
                # 2.4GHz; these throwaway matmuls (constant operands, into
                # the not-yet-used PV banks) finish before block 0 lands,
                # so the whole projection/score chain runs at full clock.
                for w in range(10):
                    nc.tensor.matmul(
                        pv_ps[w % 2][:, :],
                        lhsT=ones16[:, 0:128],
                        rhs=ones16[:],
                        start=True,
                        stop=True,
                    )

                # pre-loop: K/Q/V block 0, K/V block 1.  These run during
                # the DMA-latency window (warm thanks to the warm-up) and
                # cost only their serial time before the first ACTIVATE;
                # in-stream insertions cost ~2x their PE time in exp gaps.
                emit_kqT(0, wkc_sb, kTb[0], "k")
                emit_kqT(0, wqc_sb, qTb[0], "q")
                emit_v(0)
                if QB > 1:
                    emit_kqT(1, wkc_sb, kTb[1], "k")
                    emit_v(1)

                # K/V projection PAIRS for blocks 2..QB-1 (two ring
                # insertions preserve the 2-buffer parity).
                pair_sched = {1 + 2 * (b - 2): [("k", b), ("v", b)] for b in range(2, QB)}
                # Q prefetch for qb+1 mid-way through qb
                qpre_sched = {
                    (NS * qb + NS // 2) // NSLOT: qb + 1 for qb in range(QB - 1)
                }

                gstate = {"sc": None, "et": None, "pv_next": 0}
                slot_et = [None] * TOT  # global slot -> (exp tile, pos)

                def finish_qb(qb):
                    # normalize + transpose to natural layout + DMA out.
                    for h in range(2):
                        pvS = NRM.tile([65, 512], F32, tag=f"pvS{h}")
                        nc.vector.tensor_copy(pvS[:], pv_ps[h][0:65, :])
                        tr = pv_ps[h][:].rearrange("p (j c) -> p j c", j=4)[:, :, 0:65]
                        for c4 in range(4):
                            nc.tensor.transpose(
                                tr[:, c4, :],
                                pvS[:, 128 * c4 : 128 * (c4 + 1)],
                                ident[0:65, 0:65],
                            )
                        den, vlo = (0, 1) if h == 0 else (64, 0)
                        rec = NRM.tile([128, 4], F32, tag=f"rec{h}")
                        nc.vector.reciprocal(rec[:], tr[:, :, den])
                        for c4 in range(4):
                            j = 4 * qb + c4
                            nc.vector.tensor_scalar_mul(
                                out_sb[:, j, 64 * h : 64 * (h + 1)],
                                tr[:, c4, vlo : vlo + 64],
                                rec[:, c4 : c4 + 1],
                            )
                    nc.sync.dma_start(
                        out=out_d[512 * qb : 512 * (qb + 1), :].rearrange(
                            "(j p) c -> p j c", p=128
                        ),
                        in_=out_sb[:, 4 * qb : 4 * (qb + 1), :],
                    )

                def emit_pv(g):
                    qb, s = divmod(g, NS)
                    kt, h = divmod(s, 2)
                    et, pos = slot_et[g]
                    slot_et[g] = None
                    nc.tensor.matmul(
                        pv_ps[h][0:65, :],
                        lhsT=vb[kt // 4][:, kt % 4, 65 * h : 65 * h + 65],
                        rhs=et[:, pos, :],
                        start=(kt == 0),
                        stop=(kt == ST - 1),
                    )
                    if s == NS - 1:
                        finish_qb(qb)

                def drain_pv(upto):
                    # caller guarantees exp for slots <= upto is emitted
                    while gstate["pv_next"] <= upto:
                        emit_pv(gstate["pv_next"])
                        gstate["pv_next"] += 1

                for g in range(TOT):
                    qb, s = divmod(g, NS)
                    kt, h = divmod(s, 2)
                    pos = g % NSLOT
                    T = g // NSLOT
                    if T >= EXB:
                        # evict-time PV drain: the exp tile allocated at this
                        # T reuses the ring slot of tile T-EXB, whose slots'
                        # PV must be emitted first.  Spread the 3 drains
                        # across the tile's slots instead of bursting them
                        # at alloc (a burst delays this tile's own scores).
                        drain_pv(NSLOT * (T - EXB) + pos)
                    if pos == 0:
                        gstate["sc"] = SC.tile(
                            [128, NSLOT, 512], F32, tag="sc", name=f"sc_{g}"
                        )
                        gstate["et"] = EX.tile(
                            [128, NSLOT, 512], F16, tag="et", name=f"et_{g}"
                        )
                    nc.tensor.matmul(
                        gstate["sc"][:, pos, :],
                        lhsT=kTb[kt // 4][
                            64 * h : 64 * (h + 1), 128 * (kt % 4) : 128 * (kt % 4 + 1)
                        ],
                        rhs=qTb[qb][64 * h : 64 * (h + 1), :],
                        start=True,
                        stop=True,
                    )
                    slot_et[g] = (gstate["et"], pos)
                    if pos == NSLOT - 1 or g == TOT - 1:
                        nc.scalar.activation(
                            gstate["et"][:, : pos + 1, :],
                            gstate["sc"][:, : pos + 1, :],
                            EXPF,
                            scale=0.125,
                        )
                        if g > TOT - 2 * NS and g - tail_lag > 0:
                            # near stream end: drain eagerly so the final
                            # backlog (tail) stays small
                            drain_pv(g - tail_lag)
                        if T in pair_sched:
                            for kind, b in pair_sched[T]:
                                if kind == "k":
                                    emit_kqT(b, wkc_sb, kTb[b], "k")
                                elif kind == "q":
                                    emit_kqT(b, wqc_sb, qTb[b], "q")
                                else:
                                    emit_v(b)
                        elif T in qpre_sched:
                            emit_kqT(qpre_sched[T], wqc_sb, qTb[qpre_sched[T]], "q")
                            # parity dummy: its guard copy reads the exp
                            # tile written by the ACTIVATE above, so the
                            # ring slot's next user is gated on it (real
                            # dependency, not a timing assumption).
                            dummy = SC.tile([128, 512], F32, tag="sc", name=f"dm_{g}")
                            nc.vector.tensor_copy(
                                dummy[0:1, 0:1], gstate["et"][0:1, 0:1, 0:1]
                            )
                # all ACTIVATEs emitted: drain the tail + finish last qb
                drain_pv(TOT - 1)

    nc.compile()
    return nc


_NC_CACHE = {}

BUILD_OPTS = {"exb": 8, "tail_lag": 4}


def _get_nc(S=4096):
    key = (S, tuple(sorted(BUILD_OPTS.items())))
    if key not in _NC_CACHE:
        _NC_CACHE[key] = build_attention_nc(S=S, **BUILD_OPTS)
    return _NC_CACHE[key]


def _make_in_maps(rotation_params, entangle_params, inputs, w_q, w_k, w_v):
    B, S, E_ = inputs.shape
    assert E_ == E and B * 4 == N_CORES
    f32 = np.float32
    rot = np.asarray(rotation_params, f32)
    ent = np.asarray(entangle_params, f32)
    w_q = np.asarray(w_q, f32)
    w_k = np.asarray(w_k, f32)
    w_v = np.asarray(w_v, f32)
    # combined projection weights in f32 on host, cast fp16
    wq_comb = (w_q.T @ rot).astype(np.float16)  # [E, E]
    wk_comb = (w_k.T @ ent).astype(np.float16)
    wvT = np.ascontiguousarray(w_v.astype(np.float16).T)  # [E, E]
    xTs = [
        np.ascontiguousarray(np.asarray(inputs[b]).T.astype(np.float16))
        for b in range(B)
    ]

    def pack(w):  # [512, 128] -> [128, 512] chunk-major (1KB dram lines)
        return np.ascontiguousarray(
            w.reshape(4, 128, 128).transpose(1, 0, 2).reshape(128, 512)
        )

    in_maps = []
    for core in range(N_CORES):
        b, m = divmod(core, 4)
        cols = slice(PAIR * m, PAIR * (m + 1))
        in_maps.append(
            {
                "xT": xTs[b],
                "wqc": pack(wq_comb[:, cols]),
                "wkc": pack(wk_comb[:, cols]),
                "wvT": pack(wvT[:, cols]),
            }
        )
    return in_maps


def run(rotation_params, entangle_params, inputs, w_q, w_k, w_v, trace=False):
    """Run on the 8 NeuronCores; returns (output, BassKernelResults)."""
    inputs = np.asarray(inputs)
    B, S, E_ = inputs.shape
    nc = _get_nc(S)
    in_maps = _make_in_maps(rotation_params, entangle_params, inputs, w_q, w_k, w_v)
    res = run_bass_kernel_spmd(nc, in_maps, list(range(N_CORES)), trace=trace)
    out = np.empty((B, S, E_), dtype=np.float32)
    for core in range(N_CORES):
        b, m = divmod(core, 4)
        out[b, :, PAIR * m : PAIR * (m + 1)] = res.results[core]["out"]
    return out, res


def kernel(rotation_params, entangle_params, inputs, w_q, w_k, w_v):
    out, _ = run(rotation_params, entangle_params, inputs, w_q, w_k, w_v)
    return out
